# revision 8
# baseline (speedup 1.0000x reference)
"""DCRNN cell (diffusion-conv GRU) on 8 Trainium2 NeuronCores.

v2 strategy (graph/data parallel, 4 SPMD launches, host reassembly):
  - Target nodes sharded across 8 cores (in-degree serpentine); per-core
    node tiles of 128; schedule groups of KT=4 tiles with a degree-sorted
    active prefix per round (as v1).
  - Edge streams are SLOT-major: each 128-slot block is [slot-lane,
    feature] with slot lane == target-node lane of its tile.  The
    segment-sum runs on the TensorEngine as identity-lhsT accumulating
    matmuls into per-tile PSUM tiles (node-major accumulators), freeing
    the DVE and making every per-node norm scale a per-partition scalar
    (no [128, NPT] r-replica loads).
  - Sweep 1 streams ONE fp16 copy of each gathered source row
    (out-normalized); the in-normalized copy is derived on device by a
    per-slot q = r_in/r_out broadcast multiply (DVE+GPSIMD split).
    Exact: no fp8 noise on the dominant first-order terms.
  - Sweeps 2-4 stream fp8 (e3m4) values.  Scales S2/S3/S4 place values
    in e3m4's range; applied on device when the values are produced
    (per-partition ACT scales) and folded back into weights or copy-out
    scales.  Halves the stream bytes of v1.
  - L2/L4 need feature-major accumulators for the weight matmuls: the
    node-major PSUM accs are transposed on the TensorEngine (lhsT=acc,
    rhs=identity) -- cheap [128,128] matmuls.
  - Host does index bookkeeping, input prep/layout, per-sweep stream
    pre-gather of device-produced (already fp8) values, and shard
    reassembly.  No host feature arithmetic on intermediates.

Launches:
  L1: sweep 1 (fp16 dedup stream) -> tx1 (node-major) + t2s (fp8)
  L2: sweep 2 (fp8) + transposes + Z/R matmul -> zt, hr, t3a/t3b (fp8),
      p4x
  L3: sweep 3 (fp8) -> tx1p + t4s (fp8)
  L4: sweep 4 (fp8) + transposes + H_tilde matmul + H_new combine
"""
import numpy as np
import ml_dtypes

import concourse.bass as bass
import concourse.bacc as bacc
import concourse.tile as tile
from concourse import mybir
from concourse.bass_utils import run_bass_kernel_spmd

F32 = mybir.dt.float32
F16 = mybir.dt.float16
F8 = mybir.dt.float8e3
NP8 = ml_dtypes.float8_e3m4
ADD = mybir.AluOpType.add
MULT = mybir.AluOpType.mult
SUB = mybir.AluOpType.subtract
COPY = mybir.ActivationFunctionType.Copy
SIG = mybir.ActivationFunctionType.Sigmoid
TANH = mybir.ActivationFunctionType.Tanh

N = 50000
E = 500000
FIN = 64
FOUT = 64
C = 128          # concat dim
M = 8            # cores
TPC = 49         # tiles of 128 per core (6272 slots, 22 ghosts)
NPT = TPC * 128  # node slots per core
KT = 4           # tiles per schedule group (PSUM-sized)
CHUNK2 = 512     # L2 matmul chunk
CHUNK4 = 512     # L4 matmul chunk
CAPS = 32        # slot-tiles per stream DMA batch

# fp8 stream scales (e3m4 max 15.5; sim-verified maxima ~2.2/2.8/1.4)
S2 = 6.0
S3 = 5.0
S4 = 10.0

# Module-level knobs for test harness
TRACE = False
LAUNCH_TIMES_NS = []      # filled with per-launch exec_time_ns when TRACE


# ----------------------------------------------------------------------
# Host-side preparation
# ----------------------------------------------------------------------

def _numpy_reference(X, edge_index, H, W_z, b_z, W_r, b_r, W_h, b_h):
    """Exact numpy mirror of the jax reference (fallback path)."""
    n = X.shape[0]
    row, col = edge_index[0].astype(np.int64), edge_index[1].astype(np.int64)
    deg_out = np.bincount(row, minlength=n).astype(np.float32)
    deg_in = np.bincount(col, minlength=n).astype(np.float32)
    with np.errstate(divide="ignore"):
        norm_out = (1.0 / deg_out)[row]
        norm_in = (1.0 / deg_in)[row]
    XH = np.concatenate([X, H], axis=1)

    def prop(x, norm):
        out = np.zeros((n, x.shape[1]), np.float32)
        np.add.at(out, col, norm[:, None] * x[row])
        return out

    def dconv(Xc, W, b):
        Hout = Xc @ (W[0, 0] + W[1, 0])
        t1o = prop(Xc, norm_out)
        t1i = prop(Xc, norm_in)
        Hout = Hout + t1o @ W[0, 1] + t1i @ W[1, 1]
        t2o = 2.0 * prop(t1o, norm_out) - Xc
        t2i = 2.0 * prop(t1i, norm_in) - Xc
        Hout = Hout + t2o @ W[0, 2] + t2i @ W[1, 2]
        return Hout + b

    def sigmoid(x):
        return 1.0 / (1.0 + np.exp(-x))

    Z = sigmoid(dconv(XH, W_z, b_z))
    R = sigmoid(dconv(XH, W_r, b_r))
    XHR = np.concatenate([X, H * R], axis=1)
    Ht = np.tanh(dconv(XHR, W_h, b_h))
    Hn = Z * H + (1.0 - Z) * Ht
    mask = np.isnan(Hn)
    if mask.any():
        Hn = np.where(mask, np.nanmean(Hn), Hn)
    return Hn.astype(np.float32)


class _Prep:
    """All host-side precomputation for one input graph."""

    def __init__(self, X, edge_index, H, W_z, b_z, W_r, b_r, W_h, b_h):
        row = edge_index[0].astype(np.int64)
        col = edge_index[1].astype(np.int64)
        deg_out = np.bincount(row, minlength=N).astype(np.int64)
        deg_in = np.bincount(col, minlength=N).astype(np.int64)
        self.degenerate = bool((deg_in == 0).any() or (deg_out == 0).any())
        if self.degenerate:
            return
        r_out = (1.0 / deg_out).astype(np.float32)
        r_in = (1.0 / deg_in).astype(np.float32)
        self.r_out, self.r_in = r_out, r_in

        # --- node -> core assignment: serpentine over in-degree so every
        # 128-node tile is degree-homogeneous across cores ---
        order = np.argsort(-deg_in, kind="stable")
        node_core = np.empty(N, np.int32)
        node_lpos = np.empty(N, np.int32)
        core_nodes = np.full((M, NPT), -1, np.int64)
        nb = (N + M - 1) // M
        for b in range(nb):
            blk = order[b * M:(b + 1) * M]
            cores = range(len(blk)) if b % 2 == 0 else range(len(blk) - 1, -1, -1)
            for i, ci in enumerate(cores):
                s = blk[i]
                node_core[s] = ci
                node_lpos[s] = b
                core_nodes[ci, b] = s
        self.node_core, self.node_lpos, self.core_nodes = \
            node_core, node_lpos, core_nodes

        # --- per-(core, lpos) in-edge CSR (stable original edge order) ---
        ecore = node_core[col].astype(np.int64)
        elpos = node_lpos[col].astype(np.int64)
        key = ecore * NPT + elpos
        sidx = np.argsort(key, kind="stable")
        svals = row[sidx]                       # source gid per edge slot
        cnt = np.bincount(key, minlength=M * NPT)
        starts = np.zeros(M * NPT + 1, np.int64)
        np.cumsum(cnt, out=starts[1:])
        cnt3 = cnt.reshape(M, NPT)

        # --- schedule: group-major (group of KT tiles, round r) with a
        # degree-sorted prefix of active tiles per round ---
        Rjc = np.zeros((M, TPC), np.int64)       # per-core per-tile rounds
        for ci in range(M):
            np.maximum.at(Rjc[ci], np.arange(NPT) // 128, cnt3[ci])
        Rt = Rjc.max(axis=0)                     # cross-core rounds per tile
        groups = [(j0, min(KT, TPC - j0)) for j0 in range(0, TPC, KT)]
        rounds = [int(Rt[j0:j0 + gk].max()) for (j0, gk) in groups]
        # lightest groups first
        order = sorted(range(len(groups)), key=lambda gi: rounds[gi])
        self.groups = [groups[gi] for gi in order]
        sched = []                               # (r, j0, k)
        gentries = []                            # per group: [(r, j0, k, ko)]
        ko = 0
        for (j0, gk) in self.groups:
            Rg = int(Rt[j0:j0 + gk].max())
            ge = []
            for r in range(Rg):
                act = np.nonzero(Rt[j0:j0 + gk] > r)[0]
                k = int(act.max()) + 1 if act.size else 1
                sched.append((r, j0, k))
                ge.append((r, j0, k, ko))
                ko += k
            gentries.append(ge)
        self.sched = sched
        self.gentries = gentries
        self.totk = ko
        self.k_off = np.cumsum([0] + [k for (_, _, k) in sched])

        # per-tile round counts (entries covering tile j)
        rhat = np.zeros(TPC, np.int64)
        for (r, j0, k) in sched:
            rhat[j0:j0 + k] += 1
        self.rhat = rhat

        # --- slot sources per core: [totk*128] global src id (N = pad) ---
        S = self.totk * 128
        self.srcs = np.full((M, S), N, np.int64)
        for ci in range(M):
            for (r, j0, k), ko in zip(sched, self.k_off[:-1]):
                l = ((j0 + np.arange(k))[:, None] * 128
                     + np.arange(128)[None, :])            # [k, 128]
                d = cnt3[ci, l]
                st = starts[ci * NPT + l]
                valid = r < d
                v = svals[np.minimum(st + r, E - 1)]
                out = np.full((k, 128), N, np.int64)
                out[valid] = v[valid]
                self.srcs[ci, ko * 128:(ko + k) * 128] = out.reshape(-1)

        # --- per-slot q = r_in/r_out (for sweep-1 dedup), pad -> 1 ---
        qsrc = np.concatenate([(r_in / r_out).astype(np.float32),
                               np.ones(1, np.float32)])
        qall = qsrc[np.minimum(self.srcs, N)]              # [M, S]
        self.qtile = np.ascontiguousarray(
            qall.reshape(M, self.totk, 128).transpose(0, 2, 1)
        ).astype(np.float16)                               # [M, 128, totk]

        # --- per-node per-partition scalars [M, 128, TPC] f32 ---
        safe = np.maximum(core_nodes, 0)
        ghost = core_nodes < 0

        def nscal(pernode):
            v = pernode[safe].astype(np.float32)
            v[ghost] = 0.0
            return np.ascontiguousarray(
                v.reshape(M, TPC, 128).transpose(0, 2, 1))

        self.scal2o = nscal(2.0 * r_out * S2)
        self.scal2i = nscal(2.0 * r_in * S2)
        self.scal4o = nscal(2.0 * r_out * S4 / S3)
        self.scal4i = nscal(2.0 * r_in * S4 / S3)

        # --- per-core feature-major concat inputs ---
        Xc = np.concatenate([X.astype(np.float32), H.astype(np.float32)],
                            axis=1)                        # [N, 128]
        self.Xc = Xc
        xcs = Xc[safe]                                     # [M, NPT, 128]
        xcs[ghost] = 0.0
        self.xcs = np.ascontiguousarray(
            xcs.transpose(0, 2, 1)).astype(np.float16)     # [M, 128, NPT]
        self.hfm = np.ascontiguousarray(self.xcs[:, 64:128, :])

        def rep(v, nrows):                                 # [M, nrows, NPT]
            s = v[safe].astype(np.float32)
            s[ghost] = 0.0
            return np.ascontiguousarray(np.broadcast_to(
                s[:, None, :], (M, nrows, NPT))).astype(np.float16)

        self.r1rep_o = rep(r_out * S3, 64)                 # L2 t3a scale
        self.r1rep_i = rep(r_in * S3, 64)                  # L2 t3b scale

        # --- sweep-1 stream value table: Xc * r_out (fp16, host prep) ---
        self.v1o = (Xc * r_out[:, None]).astype(np.float16)

        # --- identities ---
        self.id16 = np.eye(128, dtype=np.float16)
        self.id8 = np.eye(128, dtype=NP8)
        self.i64 = np.eye(64, dtype=np.float16)

        # --- weights: stack Z|R in out-cols, fold Chebyshev -T0 into t0,
        # fold 1/S2 (sweep-2 scale) and 1/S4 (sweep-4) into k=2 terms ---
        W_z = W_z.astype(np.float32)
        W_r = W_r.astype(np.float32)
        W_h = W_h.astype(np.float32)

        def stk(a, b):
            return np.concatenate([a, b], axis=1)

        w1 = np.stack([
            stk(W_z[0, 0] + W_z[1, 0] - W_z[0, 2] - W_z[1, 2],
                W_r[0, 0] + W_r[1, 0] - W_r[0, 2] - W_r[1, 2]),
            stk(W_z[0, 1], W_r[0, 1]),
            stk(W_z[1, 1], W_r[1, 1]),
            stk(W_z[0, 2], W_r[0, 2]) / S2,
            stk(W_z[1, 2], W_r[1, 2]) / S2,
        ])                                                  # [5, 128, 128]
        self.w1 = w1.astype(np.float16)
        w2 = np.stack([
            W_h[0, 0] + W_h[1, 0] - W_h[0, 2] - W_h[1, 2],
            W_h[0, 1], W_h[1, 1], W_h[0, 2], W_h[1, 2],
        ])                                                  # [5, 128, 64]
        w2x = w2[:, 0:64, :].copy()
        w2x[3] /= S2
        w2x[4] /= S2
        self.w2x = np.ascontiguousarray(w2x).astype(np.float16)
        w2h = w2[:, 64:128, :].copy()
        w2h[3] /= S4
        w2h[4] /= S4
        self.w2h = np.ascontiguousarray(w2h).astype(np.float16)
        self.b1 = np.concatenate([b_z, b_r]).astype(np.float32)[:, None]
        self.b2 = b_h.astype(np.float32)[:, None]

    # -- slot-major stream build: V [N, F] -> [M, 128, totk*F]
    def slotmajor(self, V):
        F = V.shape[1]
        V1 = np.concatenate([V, np.zeros((1, F), V.dtype)])
        out = np.empty((M, 128, self.totk * F), V.dtype)
        for ci in range(M):
            G = V1[self.srcs[ci]]                  # [S, F]
            out[ci] = G.reshape(self.totk, 128, F).transpose(1, 0, 2) \
                       .reshape(128, self.totk * F)
        return out

    # -- per-core [F, NPT] feature-major shards -> per-node values [N, F]
    def unshard_fm(self, shards):
        F = shards.shape[1]
        vals = np.zeros((N, F), shards.dtype)
        for ci in range(M):
            cn = self.core_nodes[ci]
            real = cn >= 0
            vals[cn[real]] = shards[ci].T[real]
        return vals

    # -- per-core [NPT, F] node-major shards -> per-node values [N, F]
    def unshard_nm(self, shards):
        F = shards.shape[2]
        vals = np.zeros((N, F), shards.dtype)
        for ci in range(M):
            cn = self.core_nodes[ci]
            real = cn >= 0
            vals[cn[real]] = shards[ci][real]
        return vals


def _slot_batches(gentries, cap):
    """Merge consecutive entries of one group into DMA batches of at most
    `cap` slot-tiles.  Yields (ko0, ns, [(r, j0, k, ko_rel)])."""
    cur = None
    for (r, j0, k, ko) in gentries:
        if cur is not None and cur[1] + k <= cap:
            cur = (cur[0], cur[1] + k, cur[2] + [(r, j0, k, ko - cur[0])])
        else:
            if cur is not None:
                yield cur
            cur = (ko, k, [(r, j0, k, 0)])
    if cur is not None:
        yield cur


# ----------------------------------------------------------------------
# Device programs
# ----------------------------------------------------------------------

def _emit_slot_sweep(nc, prep, gi, stream_d, ident, accref, spool, F,
                     dedup=None):
    """Emit one group's stream batches + accumulating identity matmuls.

    F = stream columns per slot (128 or 256).  accref[t] = (ptile, base):
    tile t's accumulator is ptile[:, base:base+F] (PSUM tiles are
    bank-granular, so tiles share [128, 512] banks).  If
    dedup=(qt, ipool), the stream is fp16 single-copy; a q-scaled copy
    is derived per batch and accumulated into ptile[:, base+F:base+2F].
    """
    j0g, gk = prep.groups[gi]
    # one PSUM accumulation group per bank: start zeroes the whole 2KB
    # bank, so tiles sharing a bank share one start/stop chain
    mult = 2 if dedup is not None else 1
    totals = {}
    for t in range(gk):
        bk = id(accref[t][0])
        totals[bk] = totals.get(bk, 0) + int(prep.rhat[j0g + t]) * mult
    counts = {bk: 0 for bk in totals}
    for (ko0, ns, ents) in _slot_batches(prep.gentries[gi], CAPS):
        cols = ns * F
        st = spool.tile([128, CAPS * F], stream_d.dtype, tag="st")
        nc.sync.dma_start(st[:, :cols], stream_d[:, ko0 * F:(ko0 + ns) * F])
        if dedup is not None:
            qt, ipool = dedup
            sti = ipool.tile([128, CAPS * F], F16, tag="sti")
            st3 = st[:, :cols].rearrange("p (b f) -> p b f", f=F)
            o3 = sti[:, :cols].rearrange("p (b f) -> p b f", f=F)
            q3 = qt[:, ko0:ko0 + ns].unsqueeze(2).broadcast_to([128, ns, F])
            h = max(1, min(ns, round(ns * 0.62)))
            nc.vector.tensor_tensor(out=o3[:, :h], in0=st3[:, :h],
                                    in1=q3[:, :h], op=MULT)
            if h < ns:
                nc.gpsimd.tensor_tensor(out=o3[:, h:], in0=st3[:, h:],
                                        in1=q3[:, h:], op=MULT)
        flags = [(t, kr + t) for (r, j0, k, kr) in ents
                 for t in range(k)]
        for (t, bs) in flags:
            pt_, b0 = accref[t]
            bk = id(pt_)
            sa = counts[bk] == 0
            counts[bk] += 1
            nc.tensor.matmul(pt_[:, b0:b0 + F], lhsT=ident[:],
                             rhs=st[:, bs * F:(bs + 1) * F],
                             start=sa, stop=counts[bk] == totals[bk])
        if dedup is not None:
            for (t, bs) in flags:
                pt_, b0 = accref[t]
                bk = id(pt_)
                sa = counts[bk] == 0
                counts[bk] += 1
                nc.tensor.matmul(pt_[:, b0 + F:b0 + 2 * F], lhsT=ident[:],
                                 rhs=sti[:, bs * F:(bs + 1) * F],
                                 start=sa, stop=counts[bk] == totals[bk])


def _build_L1(prep):
    nc = bacc.Bacc("TRN2", target_bir_lowering=False, debug=False,
                   num_devices=M)
    tk = prep.totk
    stream_d = nc.dram_tensor("stream1", [128, tk * 128], F16,
                              kind="ExternalInput")
    q_d = nc.dram_tensor("q1", [128, tk], F16, kind="ExternalInput")
    s2o_d = nc.dram_tensor("s2o", [128, TPC], F32, kind="ExternalInput")
    s2i_d = nc.dram_tensor("s2i", [128, TPC], F32, kind="ExternalInput")
    id16_d = nc.dram_tensor("id16", [128, 128], F16, kind="ExternalInput")
    tx1_d = nc.dram_tensor("tx1", [NPT, 256], F16, kind="ExternalOutput")
    t2s_d = nc.dram_tensor("t2s", [NPT, 256], F8, kind="ExternalOutput")

    with tile.TileContext(nc) as tc:
        with tc.tile_pool(name="c", bufs=1) as cpool, \
             tc.tile_pool(name="s", bufs=3) as spool, \
             tc.tile_pool(name="i", bufs=3) as ipool, \
             tc.tile_pool(name="o", bufs=6) as opool, \
             tc.tile_pool(name="mm", bufs=4, space="PSUM") as mpool:
            qt = cpool.tile([128, tk], F16)
            nc.scalar.dma_start(qt[:], q_d[:])
            s2o = cpool.tile([128, TPC], F32)
            nc.scalar.dma_start(s2o[:], s2o_d[:])
            s2i = cpool.tile([128, TPC], F32)
            nc.scalar.dma_start(s2i[:], s2i_d[:])
            id16 = cpool.tile([128, 128], F16)
            nc.scalar.dma_start(id16[:], id16_d[:])

            for gi, (j0, gk) in enumerate(prep.groups):
                pairs = [mpool.tile([128, 512], F32, tag="acc", name="acc")
                         for _ in range((gk + 1) // 2)]
                accref = [(pairs[t // 2], (t % 2) * 256) for t in range(gk)]
                _emit_slot_sweep(nc, prep, gi, stream_d, id16, accref,
                                 spool, 128, dedup=(qt, ipool))
                for t in range(gk):
                    j = j0 + t
                    pt_, b0 = accref[t]
                    tx1s = opool.tile([128, 256], F16, tag="tx1")
                    nc.scalar.activation(tx1s[:], pt_[:, b0:b0 + 256], COPY)
                    nc.gpsimd.dma_start(tx1_d[j * 128:(j + 1) * 128, :],
                                        tx1s[:])
                    t2ss = opool.tile([128, 256], F8, tag="t2s")
                    nc.scalar.activation(t2ss[:, 0:128],
                                         pt_[:, b0:b0 + 128],
                                         COPY, scale=s2o[:, j:j + 1])
                    nc.scalar.activation(t2ss[:, 128:256],
                                         pt_[:, b0 + 128:b0 + 256],
                                         COPY, scale=s2i[:, j:j + 1])
                    nc.gpsimd.dma_start(t2s_d[j * 128:(j + 1) * 128, :],
                                        t2ss[:])
    nc.compile()
    return nc


def _build_L2(prep):
    nc = bacc.Bacc("TRN2", target_bir_lowering=False, debug=False,
                   num_devices=M)
    tk = prep.totk
    stream_d = nc.dram_tensor("stream2", [128, tk * 256], F8,
                              kind="ExternalInput")
    xcs_d = nc.dram_tensor("xcs", [128, NPT], F16, kind="ExternalInput")
    tx1_d = nc.dram_tensor("tx1", [2, 128, NPT], F16, kind="ExternalInput")
    id8_d = nc.dram_tensor("id8", [128, 128], F8, kind="ExternalInput")
    id16_d = nc.dram_tensor("id16", [128, 128], F16, kind="ExternalInput")
    w1_d = nc.dram_tensor("w1", [5, 128, 128], F16, kind="ExternalInput")
    w2x_d = nc.dram_tensor("w2x", [5, 64, 64], F16, kind="ExternalInput")
    b1z_d = nc.dram_tensor("b1z", [64, 1], F32, kind="ExternalInput")
    b1r_d = nc.dram_tensor("b1r", [64, 1], F32, kind="ExternalInput")
    r1o_d = nc.dram_tensor("r1o", [64, NPT], F16, kind="ExternalInput")
    r1i_d = nc.dram_tensor("r1i", [64, NPT], F16, kind="ExternalInput")

    zt_d = nc.dram_tensor("zt", [64, NPT], F16, kind="ExternalOutput")
    hr_d = nc.dram_tensor("hr", [64, NPT], F16, kind="ExternalOutput")
    t3a_d = nc.dram_tensor("t3a", [64, NPT], F8, kind="ExternalOutput")
    t3b_d = nc.dram_tensor("t3b", [64, NPT], F8, kind="ExternalOutput")
    p4x_d = nc.dram_tensor("p4x", [64, NPT], F16, kind="ExternalOutput")

    with tile.TileContext(nc) as tc:
        with tc.tile_pool(name="c", bufs=1) as cpool, \
             tc.tile_pool(name="s", bufs=3) as spool, \
             tc.tile_pool(name="n", bufs=8) as npool, \
             tc.tile_pool(name="o", bufs=8) as opool, \
             tc.tile_pool(name="mm", bufs=3, space="PSUM") as mpool, \
             tc.tile_pool(name="tp", bufs=2, space="PSUM") as tpool, \
             tc.tile_pool(name="pm", bufs=2, space="PSUM") as pmpool, \
             tc.tile_pool(name="p4", bufs=1, space="PSUM") as p4pool:
            xcs = cpool.tile([128, NPT], F16)
            nc.scalar.dma_start(xcs[:], xcs_d[:])
            tx1o = cpool.tile([128, NPT], F16)
            nc.scalar.dma_start(tx1o[:], tx1_d[0])
            tx1i = cpool.tile([128, NPT], F16)
            nc.scalar.dma_start(tx1i[:], tx1_d[1])
            id8 = cpool.tile([128, 128], F8)
            nc.scalar.dma_start(id8[:], id8_d[:])
            id16 = cpool.tile([128, 128], F16)
            nc.scalar.dma_start(id16[:], id16_d[:])
            w1 = cpool.tile([128, 5, 128], F16)
            for t in range(5):
                nc.scalar.dma_start(w1[:, t, :], w1_d[t])
            w2x = cpool.tile([64, 5, 64], F16)
            for t in range(5):
                nc.scalar.dma_start(w2x[:, t, :], w2x_d[t])
            b1z = cpool.tile([64, 1], F32)
            nc.scalar.dma_start(b1z[:], b1z_d[:])
            b1r = cpool.tile([64, 1], F32)
            nc.scalar.dma_start(b1r[:], b1r_d[:])
            r1o = cpool.tile([64, NPT], F16)
            nc.scalar.dma_start(r1o[:], r1o_d[:])
            r1i = cpool.tile([64, NPT], F16)
            nc.scalar.dma_start(r1i[:], r1i_d[:])
            hT = cpool.tile([64, NPT], F16)
            nc.scalar.dma_start(hT[:], xcs_d[64:128, :])
            accofm = cpool.tile([128, NPT], F16, name="accofm")
            accifm = cpool.tile([128, NPT], F16, name="accifm")

            def copy_group(j0, gk, accref):
                nms = []
                for t in range(gk):
                    pt_, b0 = accref[t]
                    nm = npool.tile([128, 256], F16, tag="nm", name="nm")
                    nc.scalar.activation(nm[:], pt_[:, b0:b0 + 256], COPY)
                    nms.append(nm)
                return nms

            def post_group(j0, gk, nms):
                for t in range(gk):
                    j = j0 + t
                    nm = nms[t]
                    tp = tpool.tile([128, 256], F32, tag="pt", name="pt")
                    nc.tensor.matmul(tp[:, 0:128], lhsT=nm[:, 0:128],
                                     rhs=id16[:], start=True, stop=True)
                    nc.scalar.activation(
                        accofm[:, j * 128:(j + 1) * 128], tp[:, 0:128], COPY)
                    nc.tensor.matmul(tp[:, 128:256], lhsT=nm[:, 128:256],
                                     rhs=id16[:], start=True, stop=True)
                    nc.scalar.activation(
                        accifm[:, j * 128:(j + 1) * 128], tp[:, 128:256],
                        COPY)
                terms = [xcs, tx1o, tx1i, accofm, accifm]
                n0 = j0 * 128
                end = n0 + gk * 128
                while n0 < end:
                    cw = min(CHUNK2, end - n0)
                    pm = pmpool.tile([128, CHUNK2], F32, tag="pm")
                    for t5 in range(5):
                        nc.tensor.matmul(pm[:, :cw], lhsT=w1[:, t5, :],
                                         rhs=terms[t5][:, n0:n0 + cw],
                                         start=(t5 == 0), stop=(t5 == 4))
                    zs = opool.tile([64, CHUNK2], F16, tag="zs")
                    nc.scalar.activation(zs[:, :cw], pm[0:64, :cw], SIG,
                                         bias=b1z[:], scale=1.0)
                    rs = opool.tile([64, CHUNK2], F16, tag="rs")
                    nc.scalar.activation(rs[:, :cw], pm[64:128, :cw], SIG,
                                         bias=b1r[:], scale=1.0)
                    nc.gpsimd.dma_start(zt_d[:, n0:n0 + cw], zs[:, :cw])
                    hrt = opool.tile([64, CHUNK2], F16, tag="hr")
                    nc.vector.tensor_tensor(hrt[:, :cw], rs[:, :cw],
                                            hT[:, n0:n0 + cw], op=MULT)
                    nc.gpsimd.dma_start(hr_d[:, n0:n0 + cw], hrt[:, :cw])
                    t3as = opool.tile([64, CHUNK2], F8, tag="t3a")
                    nc.vector.tensor_tensor(t3as[:, :cw], hrt[:, :cw],
                                            r1o[:, n0:n0 + cw], op=MULT)
                    nc.gpsimd.dma_start(t3a_d[:, n0:n0 + cw], t3as[:, :cw])
                    t3bs = opool.tile([64, CHUNK2], F8, tag="t3b")
                    nc.vector.tensor_tensor(t3bs[:, :cw], hrt[:, :cw],
                                            r1i[:, n0:n0 + cw], op=MULT)
                    nc.gpsimd.dma_start(t3b_d[:, n0:n0 + cw], t3bs[:, :cw])
                    p4 = p4pool.tile([64, CHUNK2], F32, tag="p4")
                    for t5 in range(5):
                        nc.tensor.matmul(p4[:, :cw], lhsT=w2x[:, t5, :],
                                         rhs=terms[t5][0:64, n0:n0 + cw],
                                         start=(t5 == 0), stop=(t5 == 4))
                    p4s = opool.tile([64, CHUNK2], F16, tag="p4s")
                    nc.scalar.activation(p4s[:, :cw], p4[:, :cw], COPY)
                    nc.gpsimd.dma_start(p4x_d[:, n0:n0 + cw], p4s[:, :cw])
                    n0 += cw

            prev = None
            for gi, (j0, gk) in enumerate(prep.groups):
                pairs = [mpool.tile([128, 512], F32, tag="acc", name="acc")
                         for _ in range((gk + 1) // 2)]
                accref = [(pairs[t // 2], (t % 2) * 256) for t in range(gk)]
                _emit_slot_sweep(nc, prep, gi, stream_d, id8, accref,
                                 spool, 256)
                nms = copy_group(j0, gk, accref)
                if prev is not None:
                    post_group(*prev)
                prev = (j0, gk, nms)
            post_group(*prev)
    nc.compile()
    return nc


def _build_L3(prep):
    nc = bacc.Bacc("TRN2", target_bir_lowering=False, debug=False,
                   num_devices=M)
    tk = prep.totk
    stream_d = nc.dram_tensor("stream3", [128, tk * 128], F8,
                              kind="ExternalInput")
    id8_d = nc.dram_tensor("id8", [128, 128], F8, kind="ExternalInput")
    s4o_d = nc.dram_tensor("s4o", [128, TPC], F32, kind="ExternalInput")
    s4i_d = nc.dram_tensor("s4i", [128, TPC], F32, kind="ExternalInput")
    tx1p_d = nc.dram_tensor("tx1p", [NPT, 128], F16, kind="ExternalOutput")
    t4s_d = nc.dram_tensor("t4s", [NPT, 128], F8, kind="ExternalOutput")

    with tile.TileContext(nc) as tc:
        with tc.tile_pool(name="c", bufs=1) as cpool, \
             tc.tile_pool(name="s", bufs=3) as spool, \
             tc.tile_pool(name="o", bufs=6) as opool, \
             tc.tile_pool(name="mm", bufs=2, space="PSUM") as mpool:
            id8 = cpool.tile([128, 128], F8)
            nc.scalar.dma_start(id8[:], id8_d[:])
            s4o = cpool.tile([128, TPC], F32)
            nc.scalar.dma_start(s4o[:], s4o_d[:])
            s4i = cpool.tile([128, TPC], F32)
            nc.scalar.dma_start(s4i[:], s4i_d[:])

            for gi, (j0, gk) in enumerate(prep.groups):
                quad = mpool.tile([128, 512], F32, tag="acc", name="acc")
                accref = [(quad, t * 128) for t in range(gk)]
                _emit_slot_sweep(nc, prep, gi, stream_d, id8, accref,
                                 spool, 128)
                for t in range(gk):
                    j = j0 + t
                    b0 = t * 128
                    txp = opool.tile([128, 128], F16, tag="txp")
                    nc.scalar.activation(txp[:], quad[:, b0:b0 + 128], COPY,
                                         scale=1.0 / S3)
                    nc.gpsimd.dma_start(tx1p_d[j * 128:(j + 1) * 128, :],
                                        txp[:])
                    t4 = opool.tile([128, 128], F8, tag="t4")
                    nc.scalar.activation(t4[:, 0:64], quad[:, b0:b0 + 64],
                                         COPY, scale=s4o[:, j:j + 1])
                    nc.scalar.activation(t4[:, 64:128],
                                         quad[:, b0 + 64:b0 + 128],
                                         COPY, scale=s4i[:, j:j + 1])
                    nc.gpsimd.dma_start(t4s_d[j * 128:(j + 1) * 128, :],
                                        t4[:])
    nc.compile()
    return nc


def _build_L4(prep):
    nc = bacc.Bacc("TRN2", target_bir_lowering=False, debug=False,
                   num_devices=M)
    tk = prep.totk
    stream_d = nc.dram_tensor("stream4", [128, tk * 128], F8,
                              kind="ExternalInput")
    id8_d = nc.dram_tensor("id8", [128, 128], F8, kind="ExternalInput")
    id16_d = nc.dram_tensor("id16", [128, 128], F16, kind="ExternalInput")
    i64_d = nc.dram_tensor("i64", [64, 64], F16, kind="ExternalInput")
    b2_d = nc.dram_tensor("b2", [64, 1], F32, kind="ExternalInput")
    ht_d = nc.dram_tensor("ht", [64, NPT], F16, kind="ExternalInput")
    zt_d = nc.dram_tensor("zt", [64, NPT], F16, kind="ExternalInput")
    hr_d = nc.dram_tensor("hr", [64, NPT], F16, kind="ExternalInput")
    p4x_d = nc.dram_tensor("p4x", [64, NPT], F16, kind="ExternalInput")
    u1_d = nc.dram_tensor("u1", [64, NPT], F16, kind="ExternalInput")
    u2_d = nc.dram_tensor("u2", [64, NPT], F16, kind="ExternalInput")
    w2h_d = nc.dram_tensor("w2h", [5, 64, 64], F16, kind="ExternalInput")
    out_d = nc.dram_tensor("hnew", [64, NPT], F16, kind="ExternalOutput")

    with tile.TileContext(nc) as tc:
        with tc.tile_pool(name="c", bufs=1) as cpool, \
             tc.tile_pool(name="s", bufs=3) as spool, \
             tc.tile_pool(name="n", bufs=8) as npool, \
             tc.tile_pool(name="o", bufs=6) as opool, \
             tc.tile_pool(name="mm", bufs=2, space="PSUM") as mpool, \
             tc.tile_pool(name="tp", bufs=2, space="PSUM") as tpool, \
             tc.tile_pool(name="pm", bufs=2, space="PSUM") as pmpool:
            id8 = cpool.tile([128, 128], F8)
            nc.scalar.dma_start(id8[:], id8_d[:])
            id16 = cpool.tile([128, 128], F16)
            nc.scalar.dma_start(id16[:], id16_d[:])
            i64 = cpool.tile([64, 64], F16)
            nc.scalar.dma_start(i64[:], i64_d[:])
            b2 = cpool.tile([64, 1], F32)
            nc.scalar.dma_start(b2[:], b2_d[:])
            hT = cpool.tile([64, NPT], F16)
            nc.scalar.dma_start(hT[:], ht_d[:])
            zts = cpool.tile([64, NPT], F16)
            nc.scalar.dma_start(zts[:], zt_d[:])
            hrt = cpool.tile([64, NPT], F16)
            nc.scalar.dma_start(hrt[:], hr_d[:])
            p4xs = cpool.tile([64, NPT], F16)
            nc.scalar.dma_start(p4xs[:], p4x_d[:])
            u1 = cpool.tile([64, NPT], F16)
            nc.scalar.dma_start(u1[:], u1_d[:])
            u2 = cpool.tile([64, NPT], F16)
            nc.scalar.dma_start(u2[:], u2_d[:])
            w2h = cpool.tile([64, 5, 64], F16)
            for t in range(5):
                nc.scalar.dma_start(w2h[:, t, :], w2h_d[t])
            accfa = cpool.tile([64, NPT], F16, name="accfa")
            accfb = cpool.tile([64, NPT], F16, name="accfb")

            def copy_group(j0, gk, accref):
                nms = []
                for t in range(gk):
                    pt_, b0 = accref[t]
                    nm = npool.tile([128, 128], F16, tag="nm", name="nm")
                    nc.scalar.activation(nm[:], pt_[:, b0:b0 + 128], COPY)
                    nms.append(nm)
                return nms

            def post_group(j0, gk, nms):
                for t in range(gk):
                    j = j0 + t
                    pt = tpool.tile([128, 128], F32, tag="pt", name="pt")
                    nc.tensor.matmul(pt[:], lhsT=nms[t][:], rhs=id16[:],
                                     start=True, stop=True)
                    nc.scalar.activation(
                        accfa[:, j * 128:(j + 1) * 128], pt[0:64, :], COPY)
                    nc.scalar.activation(
                        accfb[:, j * 128:(j + 1) * 128], pt[64:128, :], COPY)
                rhs5 = [hrt, u1, u2, accfa, accfb]
                n0 = j0 * 128
                end = n0 + gk * 128
                while n0 < end:
                    cw = min(CHUNK4, end - n0)
                    pm = pmpool.tile([64, CHUNK4], F32, tag="pm")
                    nc.tensor.matmul(pm[:, :cw], lhsT=i64[:],
                                     rhs=p4xs[:, n0:n0 + cw],
                                     start=True, stop=False)
                    for t5 in range(5):
                        nc.tensor.matmul(pm[:, :cw], lhsT=w2h[:, t5, :],
                                         rhs=rhs5[t5][:, n0:n0 + cw],
                                         start=False, stop=(t5 == 4))
                    hts = opool.tile([64, CHUNK4], F16, tag="hts")
                    nc.scalar.activation(hts[:, :cw], pm[:, :cw], TANH,
                                         bias=b2[:], scale=1.0)
                    d = opool.tile([64, CHUNK4], F16, tag="d")
                    nc.vector.tensor_tensor(d[:, :cw], hT[:, n0:n0 + cw],
                                            hts[:, :cw], op=SUB)
                    nc.vector.tensor_tensor(d[:, :cw], d[:, :cw],
                                            zts[:, n0:n0 + cw], op=MULT)
                    hn = opool.tile([64, CHUNK4], F16, tag="hn")
                    nc.vector.tensor_tensor(hn[:, :cw], d[:, :cw],
                                            hts[:, :cw], op=ADD)
                    nc.gpsimd.dma_start(out_d[:, n0:n0 + cw], hn[:, :cw])
                    n0 += cw

            prev = None
            for gi, (j0, gk) in enumerate(prep.groups):
                quad = mpool.tile([128, 512], F32, tag="acc", name="acc")
                accref = [(quad, t * 128) for t in range(gk)]
                _emit_slot_sweep(nc, prep, gi, stream_d, id8, accref,
                                 spool, 128)
                nms = copy_group(j0, gk, accref)
                if prev is not None:
                    post_group(*prev)
                prev = (j0, gk, nms)
            post_group(*prev)
    nc.compile()
    return nc


# ----------------------------------------------------------------------
# Runner
# ----------------------------------------------------------------------

_PROGRAM_CACHE = {}


def _run(nc, in_maps, label):
    res = run_bass_kernel_spmd(nc, in_maps, list(range(M)), trace=TRACE)
    if TRACE:
        LAUNCH_TIMES_NS.append((label, res.exec_time_ns))
    return res.results


def kernel(X, edge_index, H, W_z, b_z, W_r, b_r, W_h, b_h):
    X = np.asarray(X, np.float32)
    H = np.asarray(H, np.float32)
    edge_index = np.asarray(edge_index)
    W_z, W_r, W_h = (np.asarray(w, np.float32) for w in (W_z, W_r, W_h))
    b_z, b_r, b_h = (np.asarray(b, np.float32) for b in (b_z, b_r, b_h))

    if X.shape != (N, FIN) or edge_index.shape != (2, E):
        return _numpy_reference(X, edge_index, H, W_z, b_z, W_r, b_r,
                                W_h, b_h)

    prep = _Prep(X, edge_index, H, W_z, b_z, W_r, b_r, W_h, b_h)
    if prep.degenerate:
        return _numpy_reference(X, edge_index, H, W_z, b_z, W_r, b_r,
                                W_h, b_h)

    key = ("progs", prep.totk, tuple(prep.sched))
    if key not in _PROGRAM_CACHE:
        _PROGRAM_CACHE.clear()
        _PROGRAM_CACHE[key] = (_build_L1(prep), _build_L2(prep),
                               _build_L3(prep), _build_L4(prep))
    L1, L2, L3, L4 = _PROGRAM_CACHE[key]

    # ---- L1: sweep 1 (fp16 dedup stream)
    stream1 = prep.slotmajor(prep.v1o)
    ins = [{"stream1": stream1[ci], "q1": prep.qtile[ci],
            "s2o": prep.scal2o[ci], "s2i": prep.scal2i[ci],
            "id16": prep.id16} for ci in range(M)]
    r1 = _run(L1, ins, "L1")

    # ---- L2: sweep 2 (fp8) + Z/R
    t2s = np.stack([r1[ci]["t2s"] for ci in range(M)])   # [M, NPT, 256] fp8
    V2 = prep.unshard_nm(t2s)
    stream2 = prep.slotmajor(V2)
    tx1fm = [np.ascontiguousarray(
        r1[ci]["tx1"].T.reshape(2, 128, NPT)) for ci in range(M)]
    ins = [{"stream2": stream2[ci], "xcs": prep.xcs[ci],
            "tx1": tx1fm[ci], "id8": prep.id8, "id16": prep.id16,
            "w1": prep.w1, "w2x": prep.w2x,
            "b1z": prep.b1[:64], "b1r": prep.b1[64:],
            "r1o": prep.r1rep_o[ci], "r1i": prep.r1rep_i[ci]}
           for ci in range(M)]
    r2 = _run(L2, ins, "L2")

    # ---- L3: sweep 3 (fp8)
    v3a = prep.unshard_fm(np.stack([r2[ci]["t3a"] for ci in range(M)]))
    v3b = prep.unshard_fm(np.stack([r2[ci]["t3b"] for ci in range(M)]))
    V3 = np.concatenate([v3a, v3b], axis=1)              # [N, 128] fp8
    stream3 = prep.slotmajor(V3)
    ins = [{"stream3": stream3[ci], "id8": prep.id8,
            "s4o": prep.scal4o[ci], "s4i": prep.scal4i[ci]}
           for ci in range(M)]
    r3 = _run(L3, ins, "L3")

    # ---- L4: sweep 4 (fp8) + H_tilde + combine
    t4s = np.stack([r3[ci]["t4s"] for ci in range(M)])   # [M, NPT, 128] fp8
    V4 = prep.unshard_nm(t4s)
    stream4 = prep.slotmajor(V4)
    ins = [{"stream4": stream4[ci], "id8": prep.id8, "id16": prep.id16,
            "i64": prep.i64, "b2": prep.b2, "w2h": prep.w2h,
            "ht": prep.hfm[ci], "zt": r2[ci]["zt"], "hr": r2[ci]["hr"],
            "p4x": r2[ci]["p4x"],
            "u1": np.ascontiguousarray(r3[ci]["tx1p"].T[0:64]),
            "u2": np.ascontiguousarray(r3[ci]["tx1p"].T[64:128])}
           for ci in range(M)]
    r4 = _run(L4, ins, "L4")
    hn = np.stack([r4[ci]["hnew"] for ci in range(M)])
    H_new = prep.unshard_fm(hn).astype(np.float32)

    mask = np.isnan(H_new)
    if mask.any():
        H_new = np.where(mask, np.nanmean(H_new), H_new)
    return H_new.astype(np.float32)
